# revision 6
# baseline (speedup 1.0000x reference)
"""DOTA mix E-step (vq_codebook) on 8 TRN2 NeuronCores.

out[b,k,m] = gamma_class[b,k] * softmax_m(-0.5*(log_det+maha) + log_pi)

Math: logit[b,j] = sum_d x2[b,d]*A[d,j] + sum_d x[b,d]*B[d,j]
with A = -0.5/var + c_j (fold of the per-mode constant c_j legal since
sum_d x^2 = 1), B = mu/var, c_j = -0.5*(log_det + mu'inv mu) + log_pi.
A and B are class-mean centered over each class's valid modes: a per-class
column constant cancels in the per-class softmax, which (a) bounds logits
(max per class >= 0, so S >= 1, no max pass) and (b) shrinks A/B range so
the x2 GEMM runs in fp8-e4m3 DoubleRow mode (2x PE throughput); the x GEMM
stays fp16. Scales: x2*512 (e4m3), A*4 (e4m3), B*2048 (f16), so both GEMMs
accumulate 2048*logit in PSUM and one exp(ps/2048) dequantizes. fp8 stored
values are kept in [-240, 240]: the PE decodes 0xFE/0xFF (|x|=448 in
e4m3fn) as NaN.

Plane-major packing: classes with count>=2 sorted by mode count descending,
dealt round-robin to 8 cores. Plane m holds columns for classes with
count > m (the first n_m sorted classes), so segmented softmax sums are
contiguous slab adds S[:, :n_m] += e[:, P_m:P_m+n_m] and the gamma/S
broadcast multiply is per-plane contiguous. Count-1 classes are handled on
host (softmax of one element is 1). Per-plane SPMD pad columns store
A = -240 so exp(-60) ~ 9e-27 vanishes in S.

Post-GEMM ops are batched over groups of 4 batch chunks (tiles hold
[128, 4*width] with chunk-major layout) to amortize the ~150-250ns
per-instruction overhead of the vector/gpsimd engines.
"""

import sys

import numpy as np

sys.path.insert(0, "/opt/trn_rl_repo")

import ml_dtypes

import concourse.bass as bass
import concourse.mybir as mybir
import concourse.tile as tile
from concourse import bacc, bass_utils

F32 = mybir.dt.float32
F16 = mybir.dt.float16
F8 = mybir.dt.float8e4
NPF8 = ml_dtypes.float8_e4m3fn

B, K, M, D = 4096, 1000, 8, 512
NCORES = 8
NB = B // 128             # 32 batch chunks
GC = 4                    # chunks per post-GEMM group
EPS_REG = 1e-3
SX2 = 512.0               # x2 fp8 scale
SA = 4.0                  # A fp8 scale
SL = SX2 * SA             # = 2048: PSUM holds SL * logit
PAD_A = -240.0            # pad column: logit = -240*512/2048 = -60


def build_bass(planes):
    """planes: tuple n_m (m=0..7), columns per plane per core."""
    planes = [n for n in planes if n > 0]
    nv = sum(planes)
    kc = planes[0]
    poff = np.cumsum([0] + planes).tolist()   # plane offsets
    assert nv <= 1024
    colt8 = [(c0, min(256, nv - c0)) for c0 in range(0, nv, 256)]
    colt16 = [(c0, min(512, nv - c0)) for c0 in range(0, nv, 512)]

    nc = bacc.Bacc("TRN2", debug=False, target_bir_lowering=False)
    xt8 = nc.dram_tensor("xt8", (NB, 4, 128, 128), F8, kind="ExternalInput")
    xt16 = nc.dram_tensor("xt16", (NB, 4, 128, 128), F16, kind="ExternalInput")
    wa = nc.dram_tensor("wa", (4, 128, nv), F8, kind="ExternalInput")
    wb = nc.dram_tensor("wb", (4, 128, nv), F16, kind="ExternalInput")
    gam = nc.dram_tensor("gam", (NB, 128, kc), F32, kind="ExternalInput")
    out = nc.dram_tensor("out", (B, nv), F16, kind="ExternalOutput")
    warm = nc.dram_tensor("warm", (128, 128), F32, kind="ExternalOutput")

    xt8_ap, xt16_ap, gam_ap, out_ap = (xt8.ap(), xt16.ap(), gam.ap(), out.ap())

    # final-multiply plane split: DVE gets big planes, GpSimd the small ones
    # (GpSimd also owns the 7 segsum adds)
    mult_dve = list(range(min(6, len(planes))))
    mult_gp = list(range(len(mult_dve), len(planes)))

    with tile.TileContext(nc) as tc:
        with (
            tc.tile_pool(name="wpool", bufs=1) as wpool,
            tc.tile_pool(name="xpool", bufs=2) as xpool,
            tc.tile_pool(name="gpool", bufs=2) as gpool,
            tc.tile_pool(name="ppool", bufs=4, space="PSUM") as ppool,
            tc.tile_pool(name="epool", bufs=2) as epool,
            tc.tile_pool(name="spool", bufs=2) as spool,
            tc.tile_pool(name="opool", bufs=2) as opool,
        ):
            wat = wpool.tile([128, 4 * nv], F8, tag="wa")
            wa3 = wat[:].rearrange("p (i v) -> p i v", i=4)
            for i in range(4):
                nc.sync.dma_start(wa3[:, i, :], wa.ap()[i])
            wbt = wpool.tile([128, 4 * nv], F16, tag="wb")
            wb3 = wbt[:].rearrange("p (i v) -> p i v", i=4)
            for i in range(4):
                nc.sync.dma_start(wb3[:, i, :], wb.ap()[i])

            # HAM warmup: dummy matmuls while DMAs land so the real GEMM
            # starts at full clock
            wz = wpool.tile([128, 128], F16, tag="warmz")
            nc.vector.memset(wz[:], 0.0)
            wps = ppool.tile([128, 1024], F32, tag="ps")
            for i in range(48):
                nc.tensor.matmul(wps[:, 0:128], lhsT=wz[:], rhs=wz[:],
                                 start=True, stop=True)
            wsb = wpool.tile([128, 128], F32, tag="warmsb")
            nc.vector.tensor_copy(wsb[:], wps[:, 0:128])
            nc.sync.dma_start(warm.ap()[:, :], wsb[:])

            for g0 in range(0, NB, GC):
                x8g = xpool.tile([128, GC * 512], F8, tag="x8b")
                nc.scalar.dma_start(
                    x8g[:].rearrange("p (c i j) -> p c i j", c=GC, i=4),
                    xt8_ap[g0:g0 + GC].rearrange("c i p j -> p c i j"))
                x16g = xpool.tile([128, GC * 512], F16, tag="x16b")
                nc.scalar.dma_start(
                    x16g[:].rearrange("p (c i j) -> p c i j", c=GC, i=4),
                    xt16_ap[g0:g0 + GC].rearrange("c i p j -> p c i j"))
                gt = gpool.tile([128, GC * kc], F32, tag="gam")
                nc.sync.dma_start(
                    gt[:].rearrange("p (c k) -> p c k", c=GC),
                    gam_ap[g0:g0 + GC].rearrange("c p k -> p c k"))

                e4 = epool.tile([128, GC * nv], F32, tag="e4")
                pss = []
                for c in range(GC):
                    x8 = x8g[:, c * 512:(c + 1) * 512].rearrange(
                        "p (i j) -> p i j", i=4)
                    x16 = x16g[:, c * 512:(c + 1) * 512]
                    ps = ppool.tile([128, 1024], F32, tag="ps")
                    pss.append(ps)
                    # start=True marks the whole 2KB PSUM bank pending-zero,
                    # so only the first matmul touching a bank carries it;
                    # later tiles in the bank auto-zero their bytes on first
                    # write via the pending-zero mechanism.
                    started = set()
                    for c0, cw in colt8:
                        bank = c0 // 512
                        for g in range(2):
                            nc.tensor.matmul(
                                ps[:, c0:c0 + cw],
                                lhsT=x8[:, 2 * g:2 * g + 2, :],
                                rhs=wa3[:, 2 * g:2 * g + 2, c0:c0 + cw],
                                start=(g == 0 and bank not in started),
                                stop=False,
                                perf_mode=mybir.MatmulPerfMode.DoubleRow,
                            )
                        started.add(bank)
                    for c0, cw in colt16:
                        for k in range(4):
                            nc.tensor.matmul(
                                ps[:, c0:c0 + cw],
                                lhsT=x16[:, k * 128:(k + 1) * 128],
                                rhs=wb3[:, k, c0:c0 + cw],
                                start=False, stop=(k == 3),
                            )
                    nc.scalar.activation(
                        e4[:, c * nv:(c + 1) * nv], ps[:, 0:nv],
                        mybir.ActivationFunctionType.Exp, scale=1.0 / SL)

                ev = e4[:].rearrange("p (c v) -> p c v", c=GC)
                s4 = spool.tile([128, GC * kc], F32, tag="ssum")
                sv = s4[:].rearrange("p (c k) -> p c k", c=GC)
                n1 = planes[1] if len(planes) > 1 else 0
                if n1:
                    nc.gpsimd.tensor_tensor(
                        sv[:, :, 0:n1], ev[:, :, 0:n1],
                        ev[:, :, poff[1]:poff[1] + n1],
                        op=mybir.AluOpType.add)
                for m in range(2, len(planes)):
                    n = planes[m]
                    nc.gpsimd.tensor_tensor(
                        sv[:, :, 0:n], sv[:, :, 0:n],
                        ev[:, :, poff[m]:poff[m] + n],
                        op=mybir.AluOpType.add)

                rec = spool.tile([128, GC * kc], F32, tag="rec")
                nc.vector.reciprocal_approx_fast(rec[:], s4[:])
                coef = spool.tile([128, GC * kc], F32, tag="coef")
                nc.vector.tensor_mul(coef[:], rec[:], gt[:])
                cv = coef[:].rearrange("p (c k) -> p c k", c=GC)

                o4 = opool.tile([128, GC * nv], F16, tag="o4")
                ov = o4[:].rearrange("p (c v) -> p c v", c=GC)
                for m in mult_dve:
                    n = planes[m]
                    nc.vector.tensor_tensor(
                        ov[:, :, poff[m]:poff[m] + n],
                        ev[:, :, poff[m]:poff[m] + n],
                        cv[:, :, 0:n], op=mybir.AluOpType.mult)
                for m in mult_gp:
                    n = planes[m]
                    nc.gpsimd.tensor_tensor(
                        ov[:, :, poff[m]:poff[m] + n],
                        ev[:, :, poff[m]:poff[m] + n],
                        cv[:, :, 0:n], op=mybir.AluOpType.mult)
                nc.sync.dma_start(
                    out_ap[g0 * 128:(g0 + GC) * 128].rearrange(
                        "(c p) v -> p c v", c=GC),
                    ov)

    nc.compile()
    return nc


def _layout(mask):
    """Sort count>=2 classes by count desc, deal round-robin to cores.
    Returns (planes, per_core, ones): planes = SPMD-common plane sizes
    (n_m = max over cores of #{classes with count > m}); per_core = list of
    (class id array sorted desc, per-core real plane sizes)."""
    counts = mask.sum(-1).astype(int)               # (K,)
    multi = np.where(counts >= 2)[0]
    multi = multi[np.argsort(-counts[multi], kind="stable")]
    ones = np.where(counts == 1)[0]
    per_core = []
    for c in range(NCORES):
        ids = multi[c::NCORES]
        n_m = [int((counts[ids] > m).sum()) for m in range(M)]
        per_core.append((ids, n_m))
    planes = tuple(max(pc[1][m] for pc in per_core) for m in range(M))
    return planes, per_core, ones


def prep_inputs(x, gamma_class, mu_pad, var_pad, pi_pad, mask):
    x = np.asarray(x, np.float32)
    gamma_class = np.asarray(gamma_class, np.float32)
    mask = np.asarray(mask, bool)

    var = np.clip(np.asarray(var_pad, np.float64) + EPS_REG, 1e-8, None)
    inv = 1.0 / var
    logdet = np.log(var).sum(-1)                      # (K, M)
    muinv = np.asarray(mu_pad, np.float64) * inv
    muinvmu = (np.asarray(mu_pad, np.float64) * muinv).sum(-1)
    logpi = np.where(mask, np.log(np.asarray(pi_pad, np.float64) + 1e-10), 0.0)
    cmode = -0.5 * (logdet + muinvmu) + logpi         # (K, M)

    A = -0.5 * inv + cmode[..., None]                 # (K, M, D)
    Bw = muinv
    cnt = mask.sum(-1)[:, None, None].astype(np.float64)
    Am = np.where(mask[..., None], A, 0.0).sum(1, keepdims=True) / cnt
    Bm = np.where(mask[..., None], Bw, 0.0).sum(1, keepdims=True) / cnt
    Ac = np.clip((A - Am) * SA, -240.0, 240.0)        # stored fp8 = Ac*SA
    Bc = (Bw - Bm) * SL                               # stored f16 = Bc*SL

    planes, per_core, ones = _layout(mask)
    pl = [n for n in planes if n > 0]
    nv = sum(pl)
    kc = pl[0]
    poff = np.cumsum([0] + pl)

    x2 = np.clip(x.astype(np.float64) ** 2 * SX2, 0.0, 240.0)
    xt8 = np.ascontiguousarray(
        x2.astype(NPF8).reshape(NB, 128, 4, 128).transpose(0, 2, 3, 1))
    xt16 = np.ascontiguousarray(
        x.reshape(NB, 128, 4, 128).transpose(0, 2, 3, 1).astype(np.float16))

    in_maps = []
    for cidx in range(NCORES):
        ids, n_m = per_core[cidx]
        wa_c = np.full((nv, D), PAD_A, np.float32)
        wb_c = np.zeros((nv, D), np.float32)
        for m in range(len(pl)):
            n = n_m[m]
            if n:
                wa_c[poff[m]:poff[m] + n] = Ac[ids[:n], m]
                wb_c[poff[m]:poff[m] + n] = Bc[ids[:n], m]
        gcols = np.zeros((B, kc), np.float32)
        gcols[:, :len(ids)] = gamma_class[:, ids]
        in_maps.append({
            "xt8": xt8,
            "xt16": xt16,
            "wa": np.ascontiguousarray(
                wa_c.T.reshape(4, 128, nv).astype(NPF8)),
            "wb": np.ascontiguousarray(
                wb_c.T.reshape(4, 128, nv).astype(np.float16)),
            "gam": np.ascontiguousarray(gcols.reshape(NB, 128, kc)),
        })
    return in_maps, planes, per_core, ones


_NC_CACHE = {}


def _get_nc(planes):
    if planes not in _NC_CACHE:
        _NC_CACHE[planes] = build_bass(planes)
    return _NC_CACHE[planes]


def scatter_core(out, packed, per_core_entry, planes):
    """Scatter one core's packed (B, nv) into out (B, K, M)."""
    ids, n_m = per_core_entry
    pl = [n for n in planes if n > 0]
    poff = np.cumsum([0] + pl)
    for m in range(len(pl)):
        n = n_m[m]
        if n:
            out[:, ids[:n], m] = packed[:, poff[m]:poff[m] + n]


def kernel(x, gamma_class, mu_pad, var_pad, pi_pad, mask, _trace=False):
    in_maps, planes, per_core, ones = prep_inputs(
        x, gamma_class, mu_pad, var_pad, pi_pad, mask)
    gamma_class = np.asarray(gamma_class, np.float32)
    out = np.zeros((B, K, M), np.float32)
    if len(ones):
        out[:, ones, 0] = gamma_class[:, ones]
    if sum(planes) == 0:
        return out
    nc = _get_nc(planes)
    res = bass_utils.run_bass_kernel_spmd(
        nc, in_maps, core_ids=list(range(NCORES)), trace=_trace)
    for cidx in range(NCORES):
        packed = res.results[cidx]["out"].astype(np.float32)   # (B, nv)
        scatter_core(out, packed, per_core[cidx], planes)
    if _trace:
        kernel.last_results = res
    return out
